# revision 49
# baseline (speedup 1.0000x reference)
"""Trainium2 Bass kernel for nn_AttentionBlock (B=4, C=512, N=2048, H=8, DK=64).

Computation (see reference):
  xt = x.transpose(0,2,1)            # [B, N, C]
  qkv = xt @ Wp.T + bp               # [B, N, 3*H*DK], split per head into q,k,v
  S[b,i,j,h] = q[b,i,h,:]. k[b,j,h,:] * DK**-0.5
  P = softmax over i (the QUERY axis)
  O[b,i,h,:] = sum_j P[b,i,j,h] v[b,j,h,:]
  out = (O.reshape(b,n,H*DK) @ Wo.T + bo + xt).transpose(0,2,1)

Sharding: 8 cores = (batch b = core//2) x (head-group g = core%2, 4 heads each).
Each core writes two f16 partial resT[c_out, n] outputs (one per head pair);
host sums the four partials per batch and adds bias + residual.

v4 design (~151us vs the f16 baseline at ~196us; engine busy ~104-107us
each on ACT/DVE/Pool, PE ~58us):
  - ALL heavy matmuls are fp8e4 DoubleRow (0.5 cycles/row, K=2x128 per
    instruction, full 128 output partitions at tile_position (0,0); the ISA
    rejects DR outputs at column offset 64):
      * QKV projection: K = ct pairs of x8/w8.
      * S = k.q: K = 32x2 d-halves; q/k live as fp8 [64, 2(dh), N] tiles,
        head at partition row 32h. The projection evacuates to a [128,1024]
        fp8 staging tile (host pre-permutes wqk columns to
        [h0 d0:32|h1 d0:32|h0 d32:64|h1 d32:64]) and two SBUF->SBUF DMAs
        remap the dh blocks (DMA engines are otherwise idle).
      * PV: each head's vp is zero-padded to 128 lhsT columns so both heads
        write the full partition range (cost is free-size-based; the padded
        half is free). rhs = e8 [128, 2(jt), N] jtpair tiles.
  - exp work is routed per (jt, head) unit across engines:
      'a' ACT:  activation Exp -> fp8 e-tile + accum_out row sums
      'd' DVE:  Schraudolph i16/f16 bit-trick (tensor_scalar psum->i16),
                a 4x-mode f16 accum pass for the row sums (DVE), and the
                fp8 convert on POOL (gpsimd has no PSUM access and no
                accum_out, but SBUF-to-SBUF converts are legal).
    All routes emit fp8 e-tiles, so every PV matmul is DoubleRow. Routing
    is ~36 'a' / ~28 'd' (ROUTES), balancing ACT/DVE/Pool at ~100us each.
  - exp argument is shifted by SHIFT (cancels in softmax) to keep fp8 and
    the bit-trick in range; v is normalized as vp = v*rec*VP_SCALE in fp8.
  - PSUM: FOUR rotating "s" slots [128,1024] (all 8 banks). Four slots let
    consecutive exp units run on different engines without serializing on
    slot reuse. There is no resident PV accumulator: each PV half-pass
    (i 0:1024 / 1024:2048) is a compact replay burst over the buffered
    e8/vp8 tiles of the PREVIOUS pair (emitted as prework / tail), using an
    s-pool tile whose lifetime stays within one 4-buf pool rotation (no
    other s allocations may be emitted inside a burst).
  - PSUM evacuations (qk/v/outproj/O) are the other ACT/DVE load; Pool
    cannot touch PSUM, so it takes converts, vp builds and bulk DMA issue.
  - attention output is only ~4% of the final signal (residual dominates),
    so fp8/bit-trick error (~7% on the attention path) lands ~4e-3 final.

PSUM start/stop discipline: CoreSim zeroes lazily per 2KB region; only the
FIRST matmul touching a bank carries start=True, sub-bank chunks rely on the
pending-zero mark (see bass_interp pending-zero semantics).
"""

import os
import numpy as np
import ml_dtypes

import concourse.bass as bass
import concourse.tile as tile
from concourse import bacc, mybir
from concourse.bass_utils import run_bass_kernel_spmd

F32 = mybir.dt.float32
F16 = mybir.dt.float16
F8 = mybir.dt.float8e4
I16 = mybir.dt.int16
AF = mybir.ActivationFunctionType
ALU = mybir.AluOpType
DR = mybir.MatmulPerfMode.DoubleRow

B, C, N = 4, 512, 2048
H, DK = 8, 64
N_CORES = 8
SCALE = DK ** -0.5          # 0.125
SHIFT = -1.5                # exp(S + SHIFT): cancels in softmax, tames fp8
VP_SCALE = 256.0            # vp = v * rec * VP_SCALE kept in fp8 sweet spot
LOG2E = 1.4426950408889634
TRICK_MULT = SCALE * LOG2E * 1024.0            # psum (=8*S) -> f16 exponent
TRICK_BIAS = 15360.5 + SHIFT * LOG2E * 1024.0  # 15360 bias + shift + round

# exp routing per pair: 32 units in emission order (unit = 2*jt + h).
ROUTES = [['a'] * 6 + ['d', 'a'] * 12 + ['d', 'a'],
          ['a', 'd', 'a', 'd'] + ['d', 'a'] * 12 + ['d', 'a', 'a', 'a']]

# module-level stash so test.py can read profiling info
LAST_RESULT = None
_NC = None


def _build_nc():
    nc = bacc.Bacc("TRN2", target_bir_lowering=False, debug=False,
                   num_devices=N_CORES)

    x8 = nc.dram_tensor("x8", [C, N], F8, kind="ExternalInput").ap()
    wqk8 = nc.dram_tensor("wqk8", [C, 512], F8, kind="ExternalInput").ap()
    bqk = nc.dram_tensor("bqk", [128, 4], F32, kind="ExternalInput").ap()
    wv8 = nc.dram_tensor("wv8", [C, 256], F8, kind="ExternalInput").ap()
    bpv = nc.dram_tensor("bpv", [1, 256], F16, kind="ExternalInput").ap()
    ones = nc.dram_tensor("ones", [1, 128], F16, kind="ExternalInput").ap()
    wo = nc.dram_tensor("wo", [256, C], F16, kind="ExternalInput").ap()
    out_a = nc.dram_tensor("out_a", [C, N], F16, kind="ExternalOutput").ap()
    out_b = nc.dram_tensor("out_b", [C, N], F16, kind="ExternalOutput").ap()

    with tile.TileContext(nc) as tc:
        with (
            tc.tile_pool(name="consts", bufs=1) as consts,
            tc.tile_pool(name="qkpool", bufs=1) as qkpool,
            tc.tile_pool(name="vpool", bufs=1) as vpool,
            tc.tile_pool(name="epool", bufs=26) as epool,
            tc.tile_pool(name="etpool", bufs=4) as etpool,
            tc.tile_pool(name="vppool", bufs=26) as vppool,
            tc.tile_pool(name="outpool", bufs=6) as outpool,
            tc.tile_pool(name="smalls", bufs=24) as smalls,
            tc.tile_pool(name="psum", bufs=1, space="PSUM") as pp,
        ):
            # ---- loads ----
            # bulk data rides the Pool DGE queue (SWDGE costs ~1.2us of Pool
            # engine per DMA but keeps the SP queue short for the
            # latency-critical weights and qk remap DMAs).
            x_sb = consts.tile([128, 4, N], F8)
            xr = x8.rearrange("(ct ci) n -> ci ct n", ci=128)
            nc.gpsimd.dma_start(x_sb[:, :, 0:1024], xr[:, :, 0:1024])
            nc.gpsimd.dma_start(x_sb[:, :, 1024:2048], xr[:, :, 1024:2048])
            bpv_sb = consts.tile([1, 256], F16)
            nc.gpsimd.dma_start(bpv_sb[:], bpv[:])
            wv_sb = consts.tile([128, 4, 256], F8)
            nc.gpsimd.dma_start(wv_sb[:],
                                wv8.rearrange("(ct ci) d -> ci ct d", ci=128))
            wqk_sb = consts.tile([128, 4, 512], F8)
            nc.sync.dma_start(wqk_sb[:],
                              wqk8.rearrange("(ct ci) d -> ci ct d", ci=128))
            ones_sb = consts.tile([1, 128], F16)
            nc.sync.dma_start(ones_sb[:], ones[:])
            bqk_sb = consts.tile([128, 4], F32)
            nc.sync.dma_start(bqk_sb[:], bqk[:])
            wo_sb = consts.tile([128, 2, C], F16)

            shiftc = consts.tile([128, 1], F32)
            nc.vector.memset(shiftc[:], SHIFT)

            qk8 = [qkpool.tile([64, 2, N], F8, name=f"qk8_{i}")
                   for i in range(4)]  # [q0, k0, q1, k1]
            v_sb = vpool.tile([128, 16, 256], F16)
            o_sb = qkpool.tile([128, 2, N], F16, name="o_sb")

            # warm the ACT exp table while DMAs run
            warm = smalls.tile([1, 128], F16, tag="warm", name="warm")
            nc.scalar.activation(warm[:], ones_sb[:], AF.Exp)

            def s_slot(name="s_ps"):
                return pp.tile([128, 1024], F32, tag="s", bufs=4, name=name)

            def qk_proj(ft, blk, evac="dve"):
                # qk_sb[:, ft, blk*1024:+1024] = (wqk ft-tile).T @ x + bias
                ps = s_slot()
                for tch in range(4):
                    for t in range(2):
                        nc.tensor.matmul(
                            ps[:, tch * 256:(tch + 1) * 256],
                            lhsT=wqk_sb[:, 2 * t:2 * t + 2,
                                        ft * 128:(ft + 1) * 128],
                            rhs=x_sb[:, 2 * t:2 * t + 2,
                                     blk * 1024 + tch * 256:
                                     blk * 1024 + (tch + 1) * 256],
                            start=(t == 0 and tch % 2 == 0),
                            stop=(t == 1 and tch % 2 == 1),
                            perf_mode=DR, tile_position=(0, 0),
                            skip_group_check=True,
                        )
                tmp8 = etpool.tile([128, 1024], F8, tag="qktmp", bufs=2,
                                   name="qktmp")
                bias = bqk_sb[:, ft:ft + 1]
                if evac == "act":
                    nc.scalar.add(tmp8[:], ps[:], bias)
                else:
                    nc.vector.tensor_scalar(tmp8[:], ps[:], bias, None, ALU.add)
                # partitions: [h0 d0:32 | h1 d0:32 | h0 d32:64 | h1 d32:64]
                # -> dh-blocks are contiguous 64-partition ranges: 2 DMAs.
                # Issued on the Pool DGE queue to keep SP free for x8/out.
                dstt = qk8[ft]
                cols = slice(blk * 1024, (blk + 1) * 1024)
                nc.sync.dma_start(dstt[0:64, 0, cols], tmp8[0:64, :])
                nc.sync.dma_start(dstt[0:64, 1, cols], tmp8[64:128, :])

            def v_proj(nt, evac="dve"):
                # v_sb[:, nt] = x_tile.T @ wv + bpv -> [128 tokens, 256]
                ps = s_slot()[:, :256]
                for t in range(2):
                    nc.tensor.matmul(
                        ps[:, :],
                        lhsT=x_sb[:, 2 * t:2 * t + 2,
                                  nt * 128:(nt + 1) * 128],
                        rhs=wv_sb[:, 2 * t:2 * t + 2, :],
                        start=(t == 0), stop=False,
                        perf_mode=DR, tile_position=(0, 0),
                        skip_group_check=True,
                    )
                nc.tensor.matmul(
                    ps[:], lhsT=ones_sb[:1, :], rhs=bpv_sb[:1, :],
                    start=False, stop=True, skip_group_check=True,
                )
                if evac == "act":
                    nc.scalar.copy(v_sb[:, nt, :], ps[:])
                else:
                    nc.vector.tensor_copy(v_sb[:, nt, :], ps[:])

            def out_proj_unit2(p_, ic2, cot, engine="dve"):
                # (cot, ic2) double block: [128 c, 1024 i] -> DRAM f16
                dst = out_a if p_ == 0 else out_b
                ps = s_slot()
                for k in range(2):
                    ic = 2 * ic2 + k
                    nc.tensor.matmul(
                        ps[:, k * 512:(k + 1) * 512],
                        lhsT=wo_sb[:, p_, cot * 128:(cot + 1) * 128],
                        rhs=o_sb[:, p_, ic * 512:(ic + 1) * 512],
                        start=True, stop=True, skip_group_check=True,
                    )
                out_t = outpool.tile([128, 1024], F16, tag="outsb", name="out_t")
                if engine == "act":
                    nc.scalar.copy(out_t[:], ps[:])
                else:
                    nc.vector.tensor_copy(out_t[:], ps[:])
                nc.sync.dma_start(
                    dst[cot * 128:(cot + 1) * 128,
                        ic2 * 1024:(ic2 + 1) * 1024],
                    out_t[:])

            pv_b_tile = [None]

            def attention_pair(p_, prework=(), prologue_split=False):
                ROUTE = ROUTES[p_]
                qq = qk8[2 * p_]
                kk = qk8[2 * p_ + 1]
                e8s = {}
                vp8s = {}
                vp_count = {0: 0, 1: 0}
                emitted = [0]

                def get_tiles(jtp, h):
                    if (jtp, h) not in e8s:
                        e8s[(jtp, h)] = epool.tile([128, 2, N], F8, tag="e8",
                                                   name="e8")
                        vp8s[(jtp, h)] = vppool.tile(
                            [128, 2, 128], F8, tag=f"vp8{h}", bufs=13,
                            name="vp8")
                        vp_count[h] += 1
                        if vp_count[h] <= 13:
                            nc.gpsimd.memset(
                                vp8s[(jtp, h)][:, :, 64 * (1 - h):
                                               64 * (1 - h) + 64], 0)
                    return e8s[(jtp, h)], vp8s[(jtp, h)]

                def s_mm(jt, h, ih):
                    rp = 32 * h
                    s_ps = s_slot()
                    for c in range(4):
                        icc = 4 * ih + c
                        nc.tensor.matmul(
                            s_ps[:, c * 256:(c + 1) * 256],
                            lhsT=kk[rp:rp + 32, :, jt * 128:(jt + 1) * 128],
                            rhs=qq[rp:rp + 32, :, icc * 256:(icc + 1) * 256],
                            start=(c % 2 == 0), stop=(c % 2 == 1),
                            perf_mode=DR, tile_position=(rp, 0),
                            skip_group_check=True,
                        )
                    return s_ps

                def exp_half_a(e8, jt, h, ih):
                    s_ps = s_mm(jt, h, ih)
                    acc = smalls.tile([128, 1], F32, tag="acc",
                                      bufs=16, name="acc")
                    nc.scalar.activation(
                        e8[:, jt & 1, ih * 1024:(ih + 1) * 1024],
                        s_ps[:], AF.Exp, scale=SCALE, bias=shiftc[:],
                        accum_out=acc)
                    return acc

                def finish_unit(jtp, h, jt, rec_src):
                    rec = smalls.tile([128, 1], F32, tag="rec", bufs=24,
                                      name="rec")
                    nc.vector.reciprocal(rec[:], rec_src[:])
                    nc.gpsimd.tensor_scalar(
                        vp8s[(jtp, h)][:, jt & 1, 64 * h:64 * h + 64],
                        v_sb[:, jt, (2 * p_ + h) * 64:(2 * p_ + h + 1) * 64],
                        rec[:], VP_SCALE, ALU.mult, ALU.mult)

                def emit_unit(u):
                    jt, h = divmod(u, 2)
                    jtp = jt // 2
                    route = ROUTE[u]
                    e8, _ = get_tiles(jtp, h)
                    if route == 'a':
                        acc0 = exp_half_a(e8, jt, h, 0)
                        acc1 = exp_half_a(e8, jt, h, 1)
                        ssum = smalls.tile([128, 1], F32, tag="ssum", bufs=8,
                                           name="ssum")
                        nc.vector.tensor_add(ssum[:], acc0[:], acc1[:])
                        finish_unit(jtp, h, jt, ssum)
                    else:
                        et = etpool.tile([128, N], I16, tag="et", name="et")
                        for ih in range(2):
                            s_ps = s_mm(jt, h, ih)
                            nc.vector.tensor_scalar(
                                et[:, ih * 1024:(ih + 1) * 1024], s_ps[:],
                                TRICK_MULT, TRICK_BIAS, ALU.mult, ALU.add)
                        ssum = smalls.tile([128, 1], F32, tag="ssum", bufs=8,
                                           name="ssum")
                        scr = etpool.tile([128, N], F16, tag="scr", bufs=2,
                                          name="scr")
                        nc.vector.tensor_scalar(
                            scr[:], et[:].bitcast(F16), 1.0, None,
                            ALU.mult, ALU.add, accum_out=ssum[:])
                        nc.gpsimd.tensor_scalar(
                            e8[:, jt & 1, :], et[:].bitcast(F16), 1.0, None,
                            ALU.mult)
                        finish_unit(jtp, h, jt, ssum)

                def ensure(upto):
                    while emitted[0] <= min(upto, 31):
                        emit_unit(emitted[0])
                        emitted[0] += 1

                if prologue_split:
                    # jt0 units via ih0 halves first (need only qq/kk blk0);
                    # the b1 projections (evac on DVE - ACT is busy with the
                    # ih0 exps) ride between the half-rounds. jt1 units are
                    # routed 'd' so DVE gets work right after.
                    accs = {}
                    for u in range(6):
                        jt, h = divmod(u, 2)
                        e8, _ = get_tiles(jt // 2, h)
                        accs[u] = exp_half_a(e8, jt, h, 0)
                    qk_proj(2 * p_, 1, evac="dve")
                    qk_proj(2 * p_ + 1, 1, evac="dve")
                    v_proj(0, evac="dve")
                    v_proj(1, evac="dve")
                    v_proj(2, evac="dve")
                    for u in range(6):
                        jt, h = divmod(u, 2)
                        e8, _ = get_tiles(jt // 2, h)
                        acc1 = exp_half_a(e8, jt, h, 1)
                        ssum = smalls.tile([128, 1], F32, tag="ssum", bufs=8,
                                           name="ssum")
                        nc.vector.tensor_add(ssum[:], accs[u][:], acc1[:])
                        finish_unit(jt // 2, h, jt, ssum)
                    emitted[0] = 6

                pre = list(prework)
                if not prologue_split:
                    ensure(1)
                while pre:
                    for _ in range(2):
                        if pre:
                            pre.pop(0)()
                    if emitted[0] <= 31:
                        ensure(emitted[0])

                ensure(31)
                return e8s, vp8s

            def pv_replay_burst(e8s, vp8s, p_, half, engine):
                # One full PV pass (64 DR matmuls) for i columns
                # half*1024:(half+1)*1024, replayed from the buffered e8/vp8
                # tiles into an s-pool tile, then evacuated. Emitted as one
                # compact burst so the tile's lifetime stays within one
                # s-pool rotation (no other s allocations in between).
                o_ps = s_slot(name="o_acc")
                for jtp in range(8):
                    for h in range(2):
                        for ic in range(4 * half, 4 * half + 4):
                            nc.tensor.matmul(
                                o_ps[:, (ic - 4 * half) * 256:
                                     (ic - 4 * half + 1) * 256],
                                lhsT=vp8s[(jtp, h)][:],
                                rhs=e8s[(jtp, h)][:, :, ic * 256:(ic + 1) * 256],
                                start=(jtp == 0 and h == 0 and ic % 2 == 0),
                                stop=(jtp == 7 and h == 1),
                                perf_mode=DR, tile_position=(0, 0),
                                skip_group_check=True,
                            )
                dst = o_sb[:, p_, half * 1024:(half + 1) * 1024]
                if engine == "act":
                    nc.scalar.mul(dst, o_ps[:], 1.0 / VP_SCALE)
                else:
                    nc.vector.tensor_scalar(dst, o_ps[:],
                                            1.0 / VP_SCALE, None, ALU.mult)

            # ---- emission ----
            qk_proj(0, 0, evac="act")
            qk_proj(1, 0, evac="dve")

            pre0 = [lambda n=nt, e=("act" if nt % 2 == 0 else "dve"):
                    v_proj(n, evac=e) for nt in range(3, 16)]
            pre0 += [lambda f=ft, b=blk, e=ev: qk_proj(f, b, evac=e)
                     for (ft, blk), ev in (((2, 0), "act"), ((3, 0), "dve"),
                                           ((2, 1), "act"), ((3, 1), "dve"))]
            e8s0, vp8s0 = attention_pair(0, prework=pre0,
                                         prologue_split=True)
            nc.sync.dma_start(wo_sb[:],
                              wo.rearrange("(ko ki) m -> ki ko m", ki=128))

            # pair-1 prework: replay both PV passes of pair 0, then outproj
            pre1 = [lambda: pv_replay_burst(e8s0, vp8s0, 0, 0, "act")]
            pre1 += [lambda c=cot2, e=("dve" if cot2 % 2 == 0 else "act"):
                     out_proj_unit2(0, 0, c, e) for cot2 in range(4)]
            pre1 += [lambda: pv_replay_burst(e8s0, vp8s0, 0, 1, "act")]
            pre1 += [lambda c=cot2, e=("dve" if cot2 % 2 == 0 else "act"):
                     out_proj_unit2(0, 1, c, e) for cot2 in range(4)]
            e8s1, vp8s1 = attention_pair(1, prework=pre1)

            # tail: replay pair-1 PV passes, evacs, outproj
            tail_eng = ["dve", "act"]
            pv_replay_burst(e8s1, vp8s1, 1, 0, "act")
            for k in range(4):
                out_proj_unit2(1, 0, k, tail_eng[k % 2])
            pv_replay_burst(e8s1, vp8s1, 1, 1, "dve")
            for cot in range(4):
                out_proj_unit2(1, 1, cot, tail_eng[(cot + 1) % 2])

    nc.compile()
    return nc


def get_nc():
    global _NC
    if _NC is None:
        _NC = _build_nc()
    return _NC


def core_inputs(x, Wp, bp, core):
    """Host-side shard prep for one core: b = core//2, g = core%2."""
    b, g = divmod(core, 2)
    # f' permutation: tiles [q0 q1 | k0 k1 | q2 q3 | k2 k3] (local heads)
    idx = []
    for pair in range(2):
        for which in (0, 1):  # q tile, then k tile
            for dh in (0, 1):  # d half (DoubleRow k-tile dim)
                for lh in (2 * pair, 2 * pair + 1):
                    h = 4 * g + lh
                    base = h * 192 + which * 64 + dh * 32
                    idx.extend(range(base, base + 32))
    idx = np.asarray(idx)
    vidx = []
    for lh in range(4):
        h = 4 * g + lh
        base = h * 192 + 128
        vidx.extend(range(base, base + 64))
    vidx = np.asarray(vidx)

    f8 = ml_dtypes.float8_e4m3
    return {
        "x8": np.ascontiguousarray(x[b]).astype(f8),
        "wqk8": np.ascontiguousarray(Wp[idx, :].T).astype(f8),
        "bqk": np.ascontiguousarray(bp[idx].astype(np.float32).reshape(4, 128).T),
        "wv8": np.ascontiguousarray(Wp[vidx, :].T).astype(f8),
        "bpv": bp[vidx].astype(np.float16).reshape(1, 256),
        "ones": np.ones((1, 128), np.float16),
    }


def kernel(x, Wp, bp, Wo, bo):
    global LAST_RESULT
    x = np.asarray(x, dtype=np.float32)
    Wp = np.asarray(Wp, dtype=np.float32)
    bp = np.asarray(bp, dtype=np.float32)
    Wo = np.asarray(Wo, dtype=np.float32)
    bo = np.asarray(bo, dtype=np.float32)

    in_maps = []
    for core in range(N_CORES):
        b, g = divmod(core, 2)
        m = core_inputs(x, Wp, bp, core)
        m["wo"] = np.ascontiguousarray(
            Wo[:, 256 * g:256 * (g + 1)].T.astype(np.float16))
        in_maps.append(m)

    nc = get_nc()
    res = run_bass_kernel_spmd(
        nc, in_maps, core_ids=list(range(N_CORES)),
        trace=bool(int(os.environ.get("KERNEL_TRACE", "0"))),
    )
    LAST_RESULT = res
    result = np.empty((B, C, N), dtype=np.float32)
    for b in range(B):
        r0, r1 = res.results[2 * b], res.results[2 * b + 1]
        result[b] = (
            r0["out_a"].astype(np.float32) + r0["out_b"].astype(np.float32)
            + r1["out_a"].astype(np.float32) + r1["out_b"].astype(np.float32)
            + x[b] + bo[:, None]
        )
    return result
